# revision 17
# baseline (speedup 1.0000x reference)
# Trainium2 Bass kernel for nn_RNN (Elman RNN, tanh), 8-core data parallel.
#
# Problem (hardcoded): x [64, 1024, 256] f32, pre_state [64, 256] f32,
# W_in [256, 512], b_in [256], W_out [64, 256], b_out [64].
# Reference reshapes x (a pure memory reinterpret) to [S=1024, B=64, I=256]
# and scans: h = tanh([x_t, h] @ W_in.T + b_in); o_t = h @ W_out.T + b_out.
# Output o [1024, 64, 64].
#
# Strategy per core (8 "lanes" of the reshaped batch each):
#   Phase 1 (parallel): P = X @ W_x.T + b_in precomputed into SBUF.
#   Phase 2 (sequential, 1024 steps): one PSUM bank per step holds both
#     128-row halves of a^T [256, 8lanes] side by side ([128, 16]).
#     An identity matmul (start=True) injects P_t, then 4 bf16 weight
#     matmuls accumulate W_h @ h_{t-1}; one ACT tanh produces h_t (bf16)
#     directly into the SBUF h-history.
#   Phase 3 (parallel, per 64-step chunk): O^T = W_out @ H, bias, PE
#     transpose, DMA out. Phases 1/3 are interleaved into phase 2's
#     PE idle windows.
import sys

sys.path.insert(0, "/opt/trn_rl_repo")

import numpy as np
import ml_dtypes

import concourse.bass as bass
import concourse.mybir as mybir
import concourse.tile as tile
from concourse.bass_utils import run_bass_kernel_spmd

F32 = mybir.dt.float32
BF16 = mybir.dt.bfloat16

S, B, I, H, O = 1024, 64, 256, 256, 64
NCORES = 8
L = B // NCORES          # lanes per core = 8
CS = 64                  # steps per chunk
NCH = S // CS            # 16 chunks
ROWS = S * L             # 8192 rows per core
CROWS = CS * L           # 512 rows per chunk

_MAX_TAIL_WAITS = 1
_DONE = object()


def _patch_tile_drain():
    """This walrus build rejects >1 sem wait per instruction (CTRL and
    engine ops alike). Two patches: (a) split any scheduled instruction's
    extra waits onto preceding same-engine NoOps; (b) spill the Tile
    tail-drain's global-clock waits onto a chain of single-wait NoOps."""
    from bass_rust import ScopedClock

    if getattr(tile, "_wait_split_patched", False):
        return
    tile._wait_split_patched = True

    _orig_postorder = tile.postorder_instruction_blocks
    _counter = [0]

    def _split_waits_postorder(instructions, start_bb, output):
        for bb, insts in list(instructions.items()):
            new_list = []
            for inst in insts:
                si = getattr(inst, "sync_info", None)
                waits = list(si.on_wait) if si is not None else []
                if len(waits) > 1 and getattr(inst, "engine", None) is not None:
                    for w in waits[:-1]:
                        _counter[0] += 1
                        nop = mybir.InstNoOp(
                            name=f"I-wsplit-{_counter[0]}",
                            engine=inst.engine,
                            sync_info=mybir.SyncInfo(on_wait=[w], on_update=[]),
                            bass_nofuse=True,
                        )
                        new_list.append(nop)
                    si.on_wait = waits[-1:]
                new_list.append(inst)
            instructions[bb] = new_list
        return _orig_postorder(instructions, start_bb, output)

    tile.postorder_instruction_blocks = _split_waits_postorder

    def _drain_and_barrier(self, tick_clock, wait_clock):
        nc = self.nc
        probe = nc.sync.nop()
        wait_clock.add_sem_waits(
            probe.ins, ScopedClock({None: tick_clock.global_clock})
        )
        si = probe.ins.sync_info
        waits = list(si.on_wait) if si is not None else []
        if len(waits) > _MAX_TAIL_WAITS:
            si.on_wait = waits[:_MAX_TAIL_WAITS]
            rest = waits[_MAX_TAIL_WAITS:]
            for i in range(0, len(rest), _MAX_TAIL_WAITS):
                extra = nc.sync.nop()
                wait_clock.add_sem_waits(
                    extra.ins, ScopedClock({None: tick_clock.global_clock})
                )
                esi = extra.ins.sync_info
                esi.on_wait = rest[i : i + _MAX_TAIL_WAITS]

        nc.sync.drain()
        nc.all_engine_barrier()
        assert self.sems is not None
        popped = nc._tile_sem_poison_stack.pop()
        assert popped is self._sem_poison
        nc.clear_and_free_semaphores(list(self.sems.allocated().values()))
        nc.all_engine_barrier()

    tile.TileContext._drain_and_barrier = _drain_and_barrier


def build_nc(repeat=1):
    _patch_tile_drain()
    nc = bass.Bass("TRN2", num_devices=NCORES)

    x_d = nc.declare_dram_parameter("xs", [ROWS, I], F32, isOutput=False)
    h0_d = nc.declare_dram_parameter("h0t", [128, 2 * L], BF16, isOutput=False)
    wx_d = nc.declare_dram_parameter("wxt", [128, 512], F32, isOutput=False)
    wh_d = nc.declare_dram_parameter("wht", [128, 512], BF16, isOutput=False)
    id_d = nc.declare_dram_parameter("ident", [128, 128], F32, isOutput=False)
    wo_d = nc.declare_dram_parameter("wot", [128, 2 * O], BF16, isOutput=False)
    bi_d = nc.declare_dram_parameter("binv", [128, 2], F32, isOutput=False)
    bo_d = nc.declare_dram_parameter("boutv", [O, 1], F32, isOutput=False)
    out_d = nc.declare_dram_parameter("out", [ROWS, O], F32, isOutput=True)

    with tile.TileContext(nc) as tc:
      for _rep in range(repeat):
        with (
            tc.tile_pool(name=f"consts{_rep}", bufs=1) as consts,
            tc.tile_pool(name=f"xin{_rep}", bufs=2) as xin_pool,
            tc.tile_pool(name=f"xt{_rep}", bufs=4) as xt_pool,
            tc.tile_pool(name=f"pbuf{_rep}", bufs=NCH) as pbuf_pool,
            tc.tile_pool(name=f"hh{_rep}", bufs=NCH) as hh_pool,
            tc.tile_pool(name=f"p3s{_rep}", bufs=2) as p3s_pool,
            tc.tile_pool(name=f"p3r{_rep}", bufs=2) as p3r_pool,
            tc.tile_pool(name=f"p2ps{_rep}", bufs=3, space="PSUM") as p2_pool,
            tc.tile_pool(name=f"p1tps{_rep}", bufs=2, space="PSUM") as p1t_pool,
            tc.tile_pool(name=f"p1mps{_rep}", bufs=2, space="PSUM") as p1m_pool,
            tc.tile_pool(name=f"p3ps{_rep}", bufs=1, space="PSUM") as p3ps_pool,
        ):
            # ---- constants into SBUF. Issued on the scalar-engine HWDGE
            # path so they don't serialize behind the x-chunk DMAs on sync
            # (ACT is idle until the first tanh).
            ident = consts.tile([128, 128], F32, tag="ident")
            nc.scalar.dma_start(ident[:], id_d[:])
            wx = consts.tile([128, 512], F32, tag="wx")
            nc.scalar.dma_start(wx[:], wx_d[:])
            wh = consts.tile([128, 512], BF16, tag="wh")
            nc.scalar.dma_start(wh[:], wh_d[:])
            binv = consts.tile([128, 2], F32, tag="binv")
            nc.scalar.dma_start(binv[:], bi_d[:])
            h0 = consts.tile([128, 2 * L], BF16, tag="h0")
            nc.scalar.dma_start(h0[:], h0_d[:])
            wo = consts.tile([128, 2 * O], BF16, tag="wo")
            nc.scalar.dma_start(wo[:], wo_d[:])
            boutv = consts.tile([O, 1], F32, tag="boutv")
            nc.scalar.dma_start(boutv[:], bo_d[:])

            pbuf = [
                pbuf_pool.tile([128, CS * 2 * L], F32, tag="pb", name=f"pb{_rep}_{i}")
                for i in range(NCH)
            ]
            hh = [
                hh_pool.tile([128, CS * 2 * L], BF16, tag="hh", name=f"hh{_rep}_{i}")
                for i in range(NCH)
            ]

            def phase1_gen(c):
                """X-projection for chunk c. Yields between PE quanta."""
                r0 = c * CROWS
                xin = xin_pool.tile([128, 4, I], F32, tag="xin")
                nc.sync.dma_start(
                    xin[:], x_d[r0 : r0 + CROWS, :].rearrange("(b p) i -> p b i", p=128)
                )
                yield
                xts = []
                for kb in range(2):
                    pxt = p1t_pool.tile([128, CROWS], F32, tag="pxt")
                    for b in range(4):
                        nc.tensor.matmul(
                            pxt[:, 128 * b : 128 * (b + 1)],
                            xin[:, b, 128 * kb : 128 * (kb + 1)],
                            ident[:],
                            is_transpose=True,
                            start=(b == 0),
                            stop=(b == 3),
                        )
                        yield
                    xt = xt_pool.tile([128, CROWS], F32, tag="xt")
                    nc.vector.tensor_copy(xt[:], pxt[:])
                    xts.append(xt)
                pview = pbuf[c][:].rearrange("p (i x) -> p i x", x=2 * L)
                for jb in range(2):
                    pp = p1m_pool.tile([128, CROWS], F32, tag="pp")
                    for ka in range(2):
                        m = ka * 2 + jb
                        for s in range(4):
                            nc.tensor.matmul(
                                pp[:, 128 * s : 128 * (s + 1)],
                                wx[:, 128 * m : 128 * (m + 1)],
                                xts[ka][:, 128 * s : 128 * (s + 1)],
                                start=(ka == 0 and s == 0),
                                stop=(ka == 1 and s == 3),
                            )
                            yield
                    # bias-add copy PSUM -> Pbuf (strided dest: lanes of half jb)
                    dst = pview[:, :, jb * L : (jb + 1) * L]
                    src = pp[:].rearrange("p (i l) -> p i l", l=L)
                    nc.vector.tensor_scalar_add(dst, src, binv[:, jb : jb + 1])

            def phase3_gen(c, half):
                """Output projection + transpose + store for one 32-step
                half of chunk c (256 rows)."""
                HR = CROWS // 2  # 256 rows per half
                i0 = half * (CS // 2)
                hview = hh[c][:].rearrange("p (i x) -> p i x", x=2 * L)
                hslice = hview[:, i0 : i0 + CS // 2, :]
                pso = p3ps_pool.tile([O, HR], F32, tag="p3ps")
                nc.tensor.matmul(
                    pso[:], wo[:, 0:O], hslice[:, :, 0:L], start=True, stop=False
                )
                yield
                nc.tensor.matmul(
                    pso[:], wo[:, O : 2 * O], hslice[:, :, L : 2 * L],
                    start=False, stop=True,
                )
                yield
                ost = p3s_pool.tile([O, HR], F32, tag="ost")
                nc.vector.tensor_scalar_add(ost[:], pso[:], boutv[:])
                otr = p3r_pool.tile([128, 2 * O], F32, tag="otr")
                for b in range(2):
                    pst = p3ps_pool.tile([128, O], F32, tag="p3ps", name=f"pst{c}_{half}_{b}")
                    nc.tensor.matmul(
                        pst[:],
                        ost[:, 128 * b : 128 * (b + 1)],
                        ident[0:O, 0:O],
                        is_transpose=True,
                        start=True,
                        stop=True,
                    )
                    yield
                    nc.vector.tensor_copy(otr[:, O * b : O * (b + 1)], pst[:])
                r0 = c * CROWS + half * HR
                dram_ap = out_d[r0 : r0 + HR, :].rearrange("(b p) o -> p b o", p=128)
                nc.sync.dma_start(dram_ap, otr[:].rearrange("p (b o) -> p b o", o=O))

            # ---- chunk 0 phase 1 runs as four 128-row mini-slices so the
            # recurrence can start after the first slice instead of after
            # the whole 512-row chunk (the cold-clock pipeline is ~15us).
            xin0 = xin_pool.tile([128, 4, I], F32, tag="xin", name="xin0")
            nc.sync.dma_start(
                xin0[:], x_d[0:CROWS, :].rearrange("(b p) i -> p b i", p=128)
            )
            pview0 = pbuf[0][:].rearrange("p (i x) -> p i x", x=2 * L)

            def phase1_mini(m):
                """Project rows 128m..128m+128 of chunk 0 (steps 16m..16m+16)."""
                xtm = []
                for kb in range(2):
                    pxt = p1t_pool.tile([128, 128], F32, tag="pxt")
                    nc.tensor.matmul(
                        pxt[:], xin0[:, m, 128 * kb : 128 * (kb + 1)], ident[:],
                        is_transpose=True, start=True, stop=True,
                    )
                    yield
                    xt = xt_pool.tile([128, 128], F32, tag="xt", name=f"xt0_{m}_{kb}")
                    nc.vector.tensor_copy(xt[:], pxt[:])
                    xtm.append(xt)
                for jb in range(2):
                    pp = p1m_pool.tile([128, 128], F32, tag="pp")
                    for ka in range(2):
                        nc.tensor.matmul(
                            pp[:], wx[:, 128 * (ka * 2 + jb) : 128 * (ka * 2 + jb + 1)],
                            xtm[ka][:], start=(ka == 0), stop=(ka == 1),
                        )
                        yield
                    dst = pview0[:, 16 * m : 16 * (m + 1), jb * L : (jb + 1) * L]
                    nc.vector.tensor_scalar_add(
                        dst, pp[:].rearrange("p (i l) -> p i l", l=L),
                        binv[:, jb : jb + 1],
                    )

            for _ in phase1_mini(0):
                pass

            p1g = None
            for c in range(NCH):
                p1g = phase1_gen(c + 1) if c + 1 < NCH else None
                miniq = [phase1_mini(1), phase1_mini(2), phase1_mini(3)] if c == 0 else []
                # phase-3 work available this chunk: both halves of the
                # previous chunk; for the last chunk also its own first
                # half once its steps are done (enqueued at i == 33).
                p3q = []
                if c >= 1:
                    p3q = [phase3_gen(c - 1, 0), phase3_gen(c - 1, 1)]
                for i in range(CS):
                    if i == 0:
                        hp = h0[:] if c == 0 else hh[c - 1][:, (CS - 1) * 2 * L :]
                    else:
                        hp = hh[c][:, (i - 1) * 2 * L : i * 2 * L]
                    ps = p2_pool.tile([128, 2 * L], F32, tag="ps")
                    nc.tensor.matmul(
                        ps[:],
                        ident[:],
                        pbuf[c][:, i * 2 * L : (i + 1) * 2 * L],
                        start=True,
                        stop=False,
                    )
                    nc.tensor.matmul(
                        ps[:, 0:L], wh[:, 0:128], hp[:, 0:L], start=False, stop=False
                    )
                    nc.tensor.matmul(
                        ps[:, L : 2 * L], wh[:, 128:256], hp[:, 0:L],
                        start=False, stop=False,
                    )
                    nc.tensor.matmul(
                        ps[:, 0:L], wh[:, 256:384], hp[:, L : 2 * L],
                        start=False, stop=False,
                    )
                    nc.tensor.matmul(
                        ps[:, L : 2 * L], wh[:, 384:512], hp[:, L : 2 * L],
                        start=False, stop=True,
                    )
                    nc.scalar.activation(
                        hh[c][:, i * 2 * L : (i + 1) * 2 * L],
                        ps[:],
                        mybir.ActivationFunctionType.Tanh,
                    )
                    if c == NCH - 1 and i == 33:
                        p3q.append(phase3_gen(c, 0))
                    if i % 2 == 0:
                        if miniq:
                            if next(miniq[0], _DONE) is _DONE:
                                miniq.pop(0)
                        elif p1g is not None:
                            if next(p1g, _DONE) is _DONE:
                                p1g = None
                    else:
                        if p3q:
                            if next(p3q[0], _DONE) is _DONE:
                                p3q.pop(0)
                        elif c == 0 and p1g is not None:
                            if next(p1g, _DONE) is _DONE:
                                p1g = None
                # drain leftovers of this chunk's interleaved gens
                for g in miniq:
                    for _ in g:
                        pass
                if p1g is not None:
                    for _ in p1g:
                        pass
                for g in p3q:
                    for _ in g:
                        pass
            for _ in phase3_gen(NCH - 1, 1):
                pass

    return nc


def _prep_core_inputs(x, pre_state, W_in, b_in, W_out, b_out):
    """Host-side shard + layout prep. Returns list of in_maps per core."""
    x = np.asarray(x, np.float32)
    pre = np.asarray(pre_state, np.float32)
    W_in = np.asarray(W_in, np.float32)
    b_in = np.asarray(b_in, np.float32)
    W_out = np.asarray(W_out, np.float32)
    b_out = np.asarray(b_out, np.float32)

    xs_all = x.reshape(S, B, I)  # pure reshape, matching the reference

    Wx_T = np.ascontiguousarray(W_in[:, :I].T)   # [256 k, 256 j]
    Wh_T = np.ascontiguousarray(W_in[:, I:].T)   # [256 k, 256 j]

    def tiles4(WT, dtype):
        cols = []
        for ka in range(2):
            for jb in range(2):
                cols.append(WT[128 * ka : 128 * (ka + 1), 128 * jb : 128 * (jb + 1)])
        return np.ascontiguousarray(np.concatenate(cols, axis=1)).astype(dtype)

    wxt = tiles4(Wx_T, np.float32)                     # [128, 512] f32
    wht = tiles4(Wh_T, ml_dtypes.bfloat16)             # [128, 512] bf16
    ident = np.eye(128, dtype=np.float32)
    WoT = W_out.T                                      # [256, 64]
    wot = np.ascontiguousarray(
        np.concatenate([WoT[0:128, :], WoT[128:256, :]], axis=1)
    ).astype(ml_dtypes.bfloat16)                       # [128, 128] bf16
    binv = np.ascontiguousarray(np.stack([b_in[0:128], b_in[128:256]], axis=1))
    boutv = np.ascontiguousarray(b_out[:, None])

    in_maps = []
    for c in range(NCORES):
        lanes = slice(c * L, (c + 1) * L)
        xs_c = np.ascontiguousarray(xs_all[:, lanes, :]).reshape(ROWS, I)
        pre_c = pre[lanes, :]                          # [L, 256]
        h0t = (
            pre_c.T.reshape(2, 128, L).transpose(1, 0, 2).reshape(128, 2 * L)
        ).astype(ml_dtypes.bfloat16)
        in_maps.append(
            {
                "xs": xs_c,
                "h0t": h0t,
                "wxt": wxt,
                "wht": wht,
                "ident": ident,
                "wot": wot,
                "binv": binv,
                "boutv": boutv,
            }
        )
    return in_maps


_NC_CACHE = {}


def get_nc():
    if "nc" not in _NC_CACHE:
        _NC_CACHE["nc"] = build_nc()
    return _NC_CACHE["nc"]


def kernel(**inputs):
    nc = get_nc()
    in_maps = _prep_core_inputs(
        inputs["x"], inputs["pre_state"], inputs["W_in"], inputs["b_in"],
        inputs["W_out"], inputs["b_out"],
    )
    res = run_bass_kernel_spmd(nc, in_maps, core_ids=list(range(NCORES)))
    o = np.empty((S, B, O), np.float32)
    for c in range(NCORES):
        o[:, c * L : (c + 1) * L, :] = res.results[c]["out"].reshape(S, L, O)
    return o
